# revision 32
# baseline (speedup 1.0000x reference)
"""2-layer GCN encoder on two graphs, distributed over 8 Trainium2 NeuronCores.

Strategy
--------
Graph a -> cores 0-3, graph b -> cores 4-7. Each core owns 12,500 destination
nodes, LPT-binned (by in-degree) into 49 groups x 4 ranges x 64 node slots.
Per (group, half) the core issues ONE dma_gather of 2192 bf16 source rows
(256 B each; "half" = one of two overlapping int16 index apertures [0,32768)
and [17232,50000), with the 31% of edges in the overlap assigned to whichever
half balances load, so per-(range,half) capacity is a tight 548 ~ load+3,
enforced by a swap-repair pass after LPT binning).
Big merged calls amortize the ~1 us fixed SWDGE descriptor-generation cost
that dominated the previous per-(range,half) design, and bf16 halves gather
bytes.

Each 128-edge block feeds a [128 edges x 64 dst] norm-scaled selection matrix
(vector engine: fused is_equal*norm tensor_scalar, bf16) and a PE matmul
accumulating M^T @ S -> psum[feat, dst] (bf16 operands, fp32 accumulate; each
range's accumulation run is kept consecutive on the PE — interleaved psum
accumulation groups corrupt each other). Ranges sit at 548-slot offsets
inside the call, so three blocks per call straddle a range boundary and are
simply matmul'd twice with complementary masks. 4 ranges form a 256-column
group that flows through the dense chain W1 -> (+b1) relu -> W2 on chip (all
bf16).

The same compiled program serves both GCN layers:
  launch A: table = x, weights (W1, b1, W2)      -> emits g = relu(xW1 agg) W2
  launch B: table = g, weights ([I|0], b2, [I;0]) -> emits relu(agg(g) + b2)
Aggregation uses the identity  A_hat (x W) == (A_hat x) W  so the sparse part
always runs at 128 features. Host does index prep / unpermute only.
"""

import os
import numpy as np
import ml_dtypes

os.environ.setdefault("JAX_COMPILATION_CACHE_DIR", "/tmp/jax_cache")

import jax  # noqa: E402

try:
    jax.config.update("jax_compilation_cache_dir", "/tmp/jax_cache")
    jax.config.update("jax_persistent_cache_min_compile_time_secs", 0.0)
except Exception:
    pass

import concourse.bacc as bacc  # noqa: E402
import concourse.tile as tile  # noqa: E402
import concourse.mybir as mybir  # noqa: E402
from concourse.bass_utils import run_bass_kernel_spmd  # noqa: E402

BF16 = ml_dtypes.bfloat16

# ---- static problem geometry (hardcoded per contract) ----
N_NODES = 50000
E_EDGES = 800000
D_IN = 128
D_HID = 256

N_CORES = 8
CORES_PER_GRAPH = 4
NODES_PER_CORE = N_NODES // CORES_PER_GRAPH  # 12500

DTILE = 64                  # dst node slots per range
GROUPS = 49                 # dense groups of 4 ranges (256 dst slots)
R = GROUPS * 4              # 196 ranges (196*64 = 12544 slots >= 12500)
CAP_R = 548                 # edge-slot capacity per (range, half)
CAP_IDX = 4 * CAP_R         # 2208 gathered edge slots per dma_gather call
CAP_TILE = 2304             # gather tile slot capacity (round_up(2208, 128))
NBLK = CAP_TILE // 128      # 18 PE blocks per call tile
NCALLS = GROUPS * 2         # 98 gather calls per core-launch
IDXC = CAP_IDX // 16        # 138 int16 index columns per call

# int16 gather apertures into the 50000-row table (overlap = flexible edges)
AP0_LO, AP0_HI = 0, 32768
AP1_LO, AP1_HI = N_NODES - 32768, N_NODES  # [17232, 50000)

# (block, range) matmul plan: ranges sit at CAP_R-slot offsets, so blocks
# straddling a range boundary appear twice with complementary masks. Tile
# slots >= CAP_IDX are never gathered (pool tiles are memset once; their
# S entries are -1 -> zero weight).
PLAN = []
for rr in range(4):
    b_lo = (rr * CAP_R) // 128
    b_hi = ((rr + 1) * CAP_R - 1) // 128
    for b in range(b_lo, b_hi + 1):
        PLAN.append((b, rr))
NMM = len(PLAN)  # 21
META_PER_CALL = NMM          # one dstrel / one norm col per (call, matmul)
META_W = NCALLS * META_PER_CALL

_prog = None


def _build_program():
    nc = bacc.Bacc("TRN2", target_bir_lowering=False, num_swdge_queues=4)
    f32 = mybir.dt.float32
    bf16 = mybir.dt.bfloat16
    tbl = nc.declare_dram_parameter("tbl", [N_NODES, D_IN], bf16, isOutput=False)
    idx = nc.declare_dram_parameter("idx", [128, NCALLS * IDXC], mybir.dt.int16, isOutput=False)
    metad = nc.declare_dram_parameter("metad", [128, META_W], f32, isOutput=False)
    metan = nc.declare_dram_parameter("metan", [128, META_W], f32, isOutput=False)
    iota = nc.declare_dram_parameter("iota", [128, DTILE], bf16, isOutput=False)
    w1 = nc.declare_dram_parameter("w1", [128, D_HID], bf16, isOutput=False)
    w2 = nc.declare_dram_parameter("w2", [128, D_HID], bf16, isOutput=False)
    b1c = nc.declare_dram_parameter("b1c", [128, 2], f32, isOutput=False)
    gout = nc.declare_dram_parameter("gout", [GROUPS, 128, 256], bf16, isOutput=True)

    with tile.TileContext(nc) as tc:
        with (
            tc.tile_pool(name="res", bufs=1) as res,
            tc.tile_pool(name="mpool", bufs=6) as mp,
            tc.tile_pool(name="spool", bufs=8) as sp,
            tc.tile_pool(name="ssb", bufs=2) as ssb,
            tc.tile_pool(name="hsb", bufs=2) as hsb,
            tc.tile_pool(name="gsb", bufs=2) as gsbp,
            tc.tile_pool(name="psps", bufs=2, space="PSUM") as psps,
            tc.tile_pool(name="psh", bufs=2, space="PSUM") as psh,
            tc.tile_pool(name="psg", bufs=2, space="PSUM") as psg,
        ):
            idx_t = res.tile([128, NCALLS * IDXC], mybir.dt.int16)
            nc.sync.dma_start(idx_t[:], idx[:, :])
            metad_t = res.tile([128, META_W], f32)
            nc.sync.dma_start(metad_t[:], metad[:, :])
            metan_t = res.tile([128, META_W], f32)
            nc.sync.dma_start(metan_t[:], metan[:, :])
            iota_t = res.tile([128, DTILE], bf16)
            nc.sync.dma_start(iota_t[:], iota[:, :])
            w1t = res.tile([128, D_HID], bf16)
            nc.sync.dma_start(w1t[:], w1[:, :])
            w2t = res.tile([128, D_HID], bf16)
            nc.sync.dma_start(w2t[:], w2[:, :])
            b1t = res.tile([128, 2], f32)
            nc.sync.dma_start(b1t[:], b1c[:, :])

            iota_ap = iota_t[:, :]

            def meta_col(call, j, kind):
                # kind 0 = dstrel (f32), 1 = norm (bf16)
                t = metad_t if kind == 0 else metan_t
                return t[:, call * META_PER_CALL + j:call * META_PER_CALL + j + 1]

            # PLAN entries grouped by target range: each range's accumulation
            # run must be consecutive on the PE (interleaved psum accumulation
            # groups corrupt each other).
            by_rr = {rr: [(j, b) for j, (b, r2) in enumerate(PLAN) if r2 == rr]
                     for rr in range(4)}

            # zero the m pool once so the never-gathered tile tail
            # (slots CAP_IDX..CAP_TILE) can't hold NaN garbage
            for _ in range(6):
                mz = mp.tile([128, CAP_TILE], bf16, tag="m")
                nc.vector.memset(mz[:], 0)

            for q in range(GROUPS):
                m_tiles = []
                for h in range(2):
                    call = q * 2 + h
                    m = mp.tile([128, CAP_TILE], bf16, tag="m")
                    lo = AP0_LO if h == 0 else AP1_LO
                    hi = AP0_HI if h == 0 else AP1_HI
                    nc.gpsimd.dma_gather(
                        out_ap=m[:].rearrange("p (b e) -> p b e", e=D_IN),
                        in_ap=tbl[lo:hi, :],
                        idxs_ap=idx_t[:, call * IDXC:(call + 1) * IDXC],
                        num_idxs=CAP_IDX,
                        num_idxs_reg=CAP_IDX,
                        elem_size=D_IN,
                        single_packet=False,
                        queue_num=call % 4,
                    )
                    m_tiles.append(m)

                s_sb = ssb.tile([128, 256], bf16, tag="s_sb")
                for rr in range(4):
                    ps = psps.tile([128, DTILE], f32, tag="ps")
                    for h in range(2):
                        call = q * 2 + h
                        m = m_tiles[h]
                        for k, (j, b) in enumerate(by_rr[rr]):
                            s = sp.tile([128, DTILE], bf16, tag="s")
                            nc.vector.tensor_scalar(
                                out=s[:],
                                in0=iota_ap,
                                scalar1=meta_col(call, j, 0),
                                scalar2=meta_col(call, j, 1),
                                op0=mybir.AluOpType.is_equal,
                                op1=mybir.AluOpType.mult,
                            )
                            nc.tensor.matmul(
                                out=ps[:],
                                lhsT=m[:, b * 128:(b + 1) * 128],
                                rhs=s[:],
                                start=(h == 0 and k == 0),
                                stop=(h == 1 and k == len(by_rr[rr]) - 1),
                            )
                    nc.vector.tensor_copy(s_sb[:, rr * DTILE:(rr + 1) * DTILE], ps[:])

                h1ps = psh.tile([128, 512], f32, tag="h1ps")
                nc.tensor.matmul(out=h1ps[:, 0:256], lhsT=w1t[:, 0:128], rhs=s_sb[:], start=True, stop=True)
                nc.tensor.matmul(out=h1ps[:, 256:512], lhsT=w1t[:, 128:256], rhs=s_sb[:], start=True, stop=True)
                h1 = hsb.tile([128, 512], bf16, tag="h1")
                nc.scalar.activation(h1[:, 0:256], h1ps[:, 0:256], mybir.ActivationFunctionType.Relu, bias=b1t[:, 0:1])
                nc.scalar.activation(h1[:, 256:512], h1ps[:, 256:512], mybir.ActivationFunctionType.Relu, bias=b1t[:, 1:2])
                gps = psg.tile([128, 256], f32, tag="gps")
                nc.tensor.matmul(out=gps[:], lhsT=w2t[:, 0:128], rhs=h1[:, 0:256], start=True, stop=False)
                nc.tensor.matmul(out=gps[:], lhsT=w2t[:, 128:256], rhs=h1[:, 256:512], start=False, stop=True)
                gsb = gsbp.tile([128, 256], bf16, tag="gsb")
                nc.vector.tensor_copy(gsb[:], gps[:])
                nc.sync.dma_start(gout[q], gsb[:])

    nc.compile()
    return nc


def _get_program():
    global _prog
    if _prog is None:
        _prog = _build_program()
    return _prog


def _preprocess_graph(edge):
    """Per graph: per-core packing. Returns list of 4 core dicts + dinv."""
    src = np.asarray(edge[0], np.int64)
    dst = np.asarray(edge[1], np.int64)
    deg = np.bincount(dst, minlength=N_NODES).astype(np.float32)
    dinv = (1.0 / np.sqrt(deg + np.float32(1.0))).astype(np.float32)

    # append self loops
    selfs = np.arange(N_NODES, dtype=np.int64)
    asrc = np.concatenate([src, selfs])
    adst = np.concatenate([dst, selfs])
    anorm = (dinv[asrc] * dinv[adst]).astype(np.float32)

    cores = []
    for c in range(CORES_PER_GRAPH):
        lo, hi = c * NODES_PER_CORE, (c + 1) * NODES_PER_CORE
        emask = (adst >= lo) & (adst < hi)
        es = asrc[emask]
        ed = adst[emask] - lo
        en = anorm[emask]

        deg_n = np.bincount(ed, minlength=NODES_PER_CORE)

        # --- greedy LPT: nodes -> 196 ranges of <=64 slots, balancing load ---
        import heapq
        order = np.argsort(-deg_n, kind="stable")
        heap = [(0, r) for r in range(R)]
        heapq.heapify(heap)
        rng_of = np.empty(NODES_PER_CORE, np.int32)
        slots_used = np.zeros(R, np.int32)
        for v in order:
            load, r = heapq.heappop(heap)
            rng_of[v] = r
            slots_used[r] += 1
            load += int(deg_n[v])
            if slots_used[r] < DTILE:
                heapq.heappush(heap, (load, r))
        assert (slots_used <= DTILE).all()

        # --- half assignment: rigid by aperture, flex edges balance halves ---
        rigid0 = es < AP1_LO           # only aperture 0
        rigid1 = es >= AP0_HI          # only aperture 1
        flex = ~(rigid0 | rigid1)

        # per-node rigid/total edge counts for the capacity repair pass
        nr0 = np.bincount(ed[rigid0], minlength=NODES_PER_CORE)
        nr1 = np.bincount(ed[rigid1], minlength=NODES_PER_CORE)

        for _ in range(400):
            r0 = np.zeros(R, np.int64)
            r1 = np.zeros(R, np.int64)
            rt = np.zeros(R, np.int64)
            np.add.at(r0, rng_of, nr0)
            np.add.at(r1, rng_of, nr1)
            np.add.at(rt, rng_of, deg_n)
            need = np.maximum(np.maximum(r0, r1), (rt + 1) // 2)
            worst = int(np.argmax(need))
            if need[worst] <= CAP_R:
                break
            best = int(np.argmin(need))
            cand = np.nonzero(rng_of == worst)[0]
            vh = cand[np.argmax(deg_n[cand])]
            cand2 = np.nonzero(rng_of == best)[0]
            vl = cand2[np.argmin(deg_n[cand2])]
            rng_of[vh], rng_of[vl] = best, worst
        else:
            raise AssertionError("range capacity repair failed")

        # positions within each range (after any repair swaps)
        order2 = np.lexsort((np.arange(NODES_PER_CORE), rng_of))
        pos_of = np.empty(NODES_PER_CORE, np.int32)
        rstart = np.zeros(R + 1, np.int64)
        np.cumsum(np.bincount(rng_of, minlength=R), out=rstart[1:])
        pos_of[order2] = (np.arange(NODES_PER_CORE) - rstart[rng_of[order2]]).astype(np.int32)

        erng = rng_of[ed]
        # per range: want |h0 - h1| small
        n_tot = np.bincount(erng, minlength=R)
        n_r0 = np.bincount(erng[rigid0], minlength=R)
        n_flex = np.bincount(erng[flex], minlength=R)
        flex_to_h0 = np.clip(n_tot // 2 - n_r0, 0, n_flex)

        eh = np.zeros(len(es), np.int8)
        eh[rigid1] = 1
        # flex edges: first flex_to_h0[r] of each range -> h0, rest h1
        flex_idx = np.nonzero(flex)[0]
        forder = np.argsort(erng[flex_idx], kind="stable")
        fsorted = flex_idx[forder]
        frng = erng[fsorted]
        fstart = np.zeros(R + 1, np.int64)
        np.cumsum(np.bincount(frng, minlength=R), out=fstart[1:])
        within = np.arange(len(fsorted)) - fstart[frng]
        eh[fsorted[within >= flex_to_h0[frng]]] = 1

        # per (range, half) loads
        loads = np.zeros((R, 2), np.int64)
        np.add.at(loads, (erng, eh.astype(np.int64)), 1)
        assert loads.max() <= CAP_R, f"range-half overflow: {loads.max()}"

        # --- edge slot assembly ---
        gidx = erng.astype(np.int64) * 2 + eh  # (range, half) bucket per edge
        okey = np.lexsort((np.arange(len(es)), gidx))
        gsorted = gidx[okey]
        counts = np.bincount(gsorted, minlength=R * 2)
        starts = np.zeros(R * 2 + 1, np.int64)
        np.cumsum(counts, out=starts[1:])
        within_b = np.arange(len(es)) - starts[gsorted]
        # within-call slot (idx space, < CAP_IDX); tile space shares the
        # same slot id but a wider per-call stride (CAP_TILE)
        er = gsorted // 2
        ehh = gsorted % 2
        call_of = (er // 4) * 2 + ehh
        s_in_call = (er % 4) * CAP_R + within_b
        assert s_in_call.max() < CAP_IDX

        idx_slots = np.zeros(NCALLS * CAP_IDX, np.int64)
        dst_slots = np.full(NCALLS * CAP_TILE, -1.0, np.float32)
        nrm_slots = np.zeros(NCALLS * CAP_TILE, np.float32)
        ap_off = np.where(ehh == 1, AP1_LO, 0)
        idx_slots[call_of * CAP_IDX + s_in_call] = es[okey] - ap_off
        assert idx_slots.min() >= 0 and idx_slots.max() < 32768
        dst_slots[call_of * CAP_TILE + s_in_call] = pos_of[ed[okey]].astype(np.float32)
        nrm_slots[call_of * CAP_TILE + s_in_call] = en[okey]

        # int16 index stream: per call [CAP_IDX] -> wrap 16 partitions, tile x8
        a = idx_slots.reshape(NCALLS, IDXC, 16)
        idx16 = np.tile(
            np.ascontiguousarray(np.transpose(a, (2, 0, 1))).reshape(16, NCALLS * IDXC),
            (8, 1),
        ).astype(np.int16)

        # meta: per (call, matmul j): dstrel[128] (pos if slot's range == rr
        # else -1), norm[128]
        dst_b = dst_slots.reshape(NCALLS, NBLK, 128)
        nrm_b = nrm_slots.reshape(NCALLS, NBLK, 128)
        slot_rng = (np.arange(CAP_TILE) // CAP_R).reshape(NBLK, 128)
        metad = np.zeros((128, META_W), np.float32)
        metan = np.zeros((128, META_W), np.float32)
        for j, (b, rr) in enumerate(PLAN):
            mask = slot_rng[b] == rr
            dcol = np.where(mask, dst_b[:, b, :], -1.0)  # [NCALLS, 128]
            ncol = nrm_b[:, b, :]
            cbase = np.arange(NCALLS) * META_PER_CALL
            metad[:, cbase + j] = dcol.T
            metan[:, cbase + j] = ncol.T

        # column map: group q, col -> global node id (or -1)
        cols_map = np.full((GROUPS, 256), -1, np.int64)
        gq = rng_of // 4
        gcol = (rng_of % 4) * DTILE + pos_of
        cols_map[gq, gcol] = np.arange(lo, hi)

        cores.append({
            "idx": idx16,
            "metad": metad,
            "metan": metan,
            "cols_map": cols_map,
        })
    return cores


def _assemble(results, cores_a, cores_b):
    """Gather per-core gout into full [N, 128] float32 arrays per graph."""
    outs = []
    for g, cores in ((0, cores_a), (1, cores_b)):
        full = np.zeros((N_NODES, D_IN), np.float32)
        for c in range(CORES_PER_GRAPH):
            go = np.asarray(results[g * CORES_PER_GRAPH + c]["gout"], np.float32)
            cm = cores[c]["cols_map"]
            for q in range(GROUPS):
                valid = cm[q] >= 0
                full[cm[q][valid]] = go[q][:, valid].T
        outs.append(full)
    return outs


def _spot_check(full, tbl, edge, dinv, post, n_samples=24, tol=3e-2):
    """Verify a few random nodes of a launch output on host (numpy)."""
    src = np.asarray(edge[0], np.int64)
    dst = np.asarray(edge[1], np.int64)
    rng = np.random.default_rng(12345)
    nodes = rng.integers(0, N_NODES, size=n_samples)
    for v in nodes:
        ine = np.where(dst == v)[0]
        s = (dinv[src[ine]] * dinv[v])[:, None] * tbl[src[ine]]
        s = s.sum(axis=0, dtype=np.float64) + np.float64(dinv[v]) ** 2 * tbl[v]
        exp = post(s)
        got = full[v]
        scale = max(np.abs(exp).max(), 1e-3)
        if np.abs(got - exp).max() / scale > tol:
            return False
    return True


def _pack_w2(W2):
    """[256,128] -> [128, 256] lhsT-halves layout."""
    out = np.empty((128, 256), np.float32)
    out[:, 0:128] = W2[0:128, :]
    out[:, 128:256] = W2[128:256, :]
    return out


def kernel(x_a, edge_a, x_b, edge_b, W1, b1, W2, b2):
    x_a = np.ascontiguousarray(np.asarray(x_a, np.float32))
    x_b = np.ascontiguousarray(np.asarray(x_b, np.float32))
    W1 = np.asarray(W1, np.float32)
    b1 = np.asarray(b1, np.float32)
    W2 = np.asarray(W2, np.float32)
    b2 = np.asarray(b2, np.float32)

    nc = _get_program()
    cores_a = _preprocess_graph(np.asarray(edge_a))
    cores_b = _preprocess_graph(np.asarray(edge_b))

    b1c = np.stack([b1[0:128], b1[128:256]], axis=1).astype(np.float32)
    eye = np.eye(128, dtype=np.float32)
    w1_id = np.concatenate([eye, np.zeros((128, 128), np.float32)], axis=1)
    w2_id = _pack_w2(np.concatenate([eye, np.zeros((128, 128), np.float32)], axis=0))
    b1c_id = np.stack([b2, np.zeros(128, np.float32)], axis=1).astype(np.float32)

    iota_np = np.broadcast_to(
        np.arange(DTILE, dtype=np.float32), (128, DTILE)
    ).astype(BF16)

    def maps(tbl_a, tbl_b, w1m, w2m, b1m):
        w1b = w1m.astype(BF16)
        w2b = w2m.astype(BF16)
        ms = []
        for g, (tbl, cores) in enumerate(((tbl_a, cores_a), (tbl_b, cores_b))):
            for c in range(CORES_PER_GRAPH):
                ms.append({
                    "tbl": tbl,
                    "idx": cores[c]["idx"],
                    "metad": cores[c]["metad"],
                    "metan": cores[c]["metan"],
                    "iota": iota_np,
                    "w1": w1b, "w2": w2b, "b1c": b1m,
                })
        return ms

    core_ids = list(range(N_CORES))

    def run(in_maps):
        import time as _t
        last = None
        for attempt in range(4):
            try:
                t0 = _t.time()
                res = run_bass_kernel_spmd(nc, in_maps, core_ids)
                LAUNCH_WALL.append(_t.time() - t0)
                return res
            except Exception as e:  # wedged core recovers on retry
                last = e
                _t.sleep(5)
        raise last

    dinv_a = (1.0 / np.sqrt(np.bincount(np.asarray(edge_a[1], np.int64),
              minlength=N_NODES).astype(np.float32) + 1.0)).astype(np.float32)
    dinv_b = (1.0 / np.sqrt(np.bincount(np.asarray(edge_b[1], np.int64),
              minlength=N_NODES).astype(np.float32) + 1.0)).astype(np.float32)

    def post_a(s):
        return np.maximum(s @ W1.astype(np.float64) + b1, 0.0) @ W2.astype(np.float64)

    def post_b(s):
        return np.maximum(s + b2, 0.0)

    xa16 = x_a.astype(BF16)
    xb16 = x_b.astype(BF16)
    w2p = _pack_w2(W2)

    # run each launch until the host spot-check passes (guards against rare
    # silent device-side corruption)
    for attempt in range(4):
        resA = run(maps(xa16, xb16, W1, w2p, b1c))
        g_a, g_b = _assemble(resA.results, cores_a, cores_b)
        if (_spot_check(g_a, x_a, edge_a, dinv_a, post_a)
                and _spot_check(g_b, x_b, edge_b, dinv_b, post_a)):
            break
    ga16 = g_a.astype(BF16)
    gb16 = g_b.astype(BF16)
    for attempt in range(4):
        resB = run(maps(ga16, gb16, w1_id, w2_id, b1c_id))
        z_a, z_b = _assemble(resB.results, cores_a, cores_b)
        if (_spot_check(z_a, g_a, edge_a, dinv_a, post_b)
                and _spot_check(z_b, g_b, edge_b, dinv_b, post_b)):
            break
    return (z_a, z_b)


LAUNCH_WALL = []
